# revision 4
# baseline (speedup 1.0000x reference)
"""Masked cosine-similarity attention scores on 8 trn2 NeuronCores.

Problem (per full inputs):
    query [B=4, Sq=2048, 1, D=1024] f32
    key   [B=4, 1, Sk=2048, D=1024] f32
    mask  [B=4, Sk=2048] int32 (0/1)
    out[b,q,k] = (q.k)/(max(|q|,eps)*max(|k|,eps)),  -1e9 where mask[b,k]==0

Sharding: 8 cores = (batch b, q-half h); each core computes the TRANSPOSED
output tile out_T [Sk=2048, Sq_loc=1024] for its (b, h).

Per-core device algorithm (all math on device):
  - Q^T, K^T arrive in [D, rows] bf16 layout (host does layout/dtype prep only).
  - row norms^2 via squares + ones-matmul partition reduction on the PE.
  - 1/norm via ACT Sqrt + DVE reciprocal + one Newton step (rel err ~1e-5).
  - q-scales broadcast to all partitions via ones outer-product matmul, then
    folded into Q^T on the DVE.
  - main matmul: out_T[k,q] accumulated over 8 d-chunks in PSUM (bf16 PE).
  - k-scale and mask bias (-1e9) fused into the PSUM->SBUF eviction as
    ACT activation(Identity, scale=s_k[P,1], bias=bias[P,1]); masked entries
    come out as exactly -1e9 in fp32.
"""

import os
import sys

import numpy as np

for _p in ("/opt/trn_rl_repo", "/opt/pypackages"):
    if _p not in sys.path and os.path.isdir(_p):
        sys.path.append(_p)

import ml_dtypes  # noqa: E402

_NC_CACHE = {}

# Full-problem constants (hardcoded per harness contract)
B, SQ_FULL, SK, D = 4, 2048, 2048, 1024
N_CORES = 8
SQ = SQ_FULL * B // N_CORES  # 1024 local q rows per core
P = 128


def build_nc(SQ=SQ, SK=SK, D=D, QH=512):
    """Build the single-core Bass program (SPMD: same program, per-core data)."""
    import concourse.mybir as mybir
    from concourse import bacc
    from concourse.alu_op_type import AluOpType
    from concourse.tile import TileContext

    f32 = mybir.dt.float32
    bf16 = mybir.dt.bfloat16
    AF = mybir.ActivationFunctionType

    ND = D // P       # d-chunks
    NKT = SK // P     # k-tiles (output partition tiles)
    NQH = SQ // QH    # q column chunks
    NKC = SK // QH    # k column chunks for norm reduce

    nc = bacc.Bacc("TRN2", target_bir_lowering=False, debug=False)
    qt_d = nc.declare_dram_parameter("qt", [D, SQ], bf16, isOutput=False)
    kt_d = nc.declare_dram_parameter("kt", [D, SK], bf16, isOutput=False)
    mk_d = nc.declare_dram_parameter("maskf", [SK], f32, isOutput=False)
    out_d = nc.declare_dram_parameter("out", [SK, SQ], f32, isOutput=True)
    nkb_d = nc.dram_tensor("nk_bounce", [SK], f32)

    with TileContext(nc) as tc:
        with (
            tc.tile_pool(name="pp", bufs=1) as pp,
            tc.tile_pool(name="rows", bufs=3) as rows,
            tc.tile_pool(name="outp", bufs=4) as outp,
            tc.tile_pool(name="psA", bufs=2, space="PSUM") as psA,
            tc.tile_pool(name="psK", bufs=2, space="PSUM") as psK,
            tc.tile_pool(name="pso", bufs=4, space="PSUM") as pso,
        ):
            # ---- constants ----
            ones_bf = pp.tile([P, 1], bf16, name="ones_bf")
            nc.vector.memset(ones_bf[:], 1.0)
            ones_f1 = pp.tile([1, P], f32, name="ones_f1")
            nc.vector.memset(ones_f1[:], 1.0)

            # ---- input DMAs ----
            qt_ch = []
            for d in range(ND):
                t = pp.tile([P, SQ], bf16, name=f"qtc{d}", tag=f"qtc{d}")
                nc.sync.dma_start(t[:], qt_d[d * P:(d + 1) * P, :])
                qt_ch.append(t)
            kt_ch = []
            for d in range(ND):
                t = pp.tile([P, SK], bf16, name=f"ktc{d}", tag=f"ktc{d}")
                nc.sync.dma_start(t[:], kt_d[d * P:(d + 1) * P, :])
                kt_ch.append(t)
            maskc = pp.tile([P, NKT], f32, name="maskc")
            nc.sync.dma_start(maskc[:], mk_d.rearrange("(j p) -> p j", p=P))

            # ---- q norms: squares (DVE) + ones-matmul partition reduce (PE) ----
            qsq_ch = []
            for d in range(ND):
                t = pp.tile([P, SQ], bf16, name=f"qsq{d}", tag=f"qsq{d}")
                nc.vector.tensor_mul(t[:], qt_ch[d][:], qt_ch[d][:])
                qsq_ch.append(t)
            nq2r = rows.tile([1, SQ], f32, name="nq2r", tag="qrowA")
            for j in range(NQH):
                npq = psA.tile([1, QH], f32, name="npq", tag="psA")
                for d in range(ND):
                    nc.tensor.matmul(
                        npq[:], ones_bf[:], qsq_ch[d][:, j * QH:(j + 1) * QH],
                        start=(d == 0), stop=(d == ND - 1))
                nc.vector.tensor_copy(nq2r[0:1, j * QH:(j + 1) * QH], npq[:])

            # ---- s_q = rsqrt(nq2) with one Newton step (on partition-0 rows) ----
            sq_y = rows.tile([1, SQ], f32, name="sq_y", tag="qrowB")
            nc.scalar.sqrt(sq_y[:], nq2r[:])
            sq_r0 = rows.tile([1, SQ], f32, name="sq_r0", tag="qrowC")
            nc.vector.reciprocal(sq_r0[:], sq_y[:])
            sq_t1 = rows.tile([1, SQ], f32, name="sq_t1", tag="qrowB")
            nc.vector.tensor_mul(sq_t1[:], sq_r0[:], sq_r0[:])
            sq_t2 = rows.tile([1, SQ], f32, name="sq_t2", tag="qrowA")
            nc.vector.tensor_mul(sq_t2[:], sq_t1[:], nq2r[:])
            sq_t3 = rows.tile([1, SQ], f32, name="sq_t3", tag="qrowB")
            nc.vector.tensor_scalar(
                sq_t3[:], sq_t2[:], -0.5, 1.5, AluOpType.mult, AluOpType.add)
            sq_row = rows.tile([1, SQ], f32, name="sq_row", tag="qrowA")
            nc.vector.tensor_mul(sq_row[:], sq_t3[:], sq_r0[:])

            # ---- broadcast s_q across partitions, fold into Q^T ----
            sq_bc = pp.tile([P, SQ], bf16, name="sq_bc")
            for j in range(NQH):
                pb = psA.tile([P, QH], f32, name="pb", tag="psA")
                nc.tensor.matmul(
                    pb[:], ones_f1[:], sq_row[0:1, j * QH:(j + 1) * QH],
                    start=True, stop=True)
                nc.vector.tensor_copy(sq_bc[:, j * QH:(j + 1) * QH], pb[:])
            qts_ch = []
            for d in range(ND):
                t = pp.tile([P, SQ], bf16, name=f"qts{d}", tag=f"qts{d}")
                nc.vector.tensor_mul(t[:], qt_ch[d][:], sq_bc[:])
                qts_ch.append(t)

            # ---- k norms: squares (ACT) + ones-matmul reduce (PE) ----
            ksq_ch = []
            for d in range(ND):
                t = pp.tile([P, SK], bf16, name=f"ksq{d}", tag=f"ksq{d}")
                nc.scalar.square(t[:], kt_ch[d][:])
                ksq_ch.append(t)
            # Row norms^2 per 512-chunk -> DRAM bounce -> column layout.
            # Each DMA is kept to a single producer (HW limit on sync waits).
            nk2c = pp.tile([P, NKT], f32, name="nk2c")
            JPC = QH // P  # columns of nk2c covered per 512-chunk
            for j in range(NKC):
                npk = psK.tile([1, QH], f32, name="npk", tag="psK")
                for d in range(ND):
                    nc.tensor.matmul(
                        npk[:], ones_bf[:], ksq_ch[d][:, j * QH:(j + 1) * QH],
                        start=(d == 0), stop=(d == ND - 1))
                nkr = rows.tile([1, QH], f32, name="nkr", tag=f"krow{j}")
                nc.vector.tensor_copy(nkr[:], npk[:])
                nc.sync.dma_start(nkb_d[j * QH:(j + 1) * QH], nkr[0:1, :])
                nc.sync.dma_start(
                    nk2c[:, j * JPC:(j + 1) * JPC],
                    nkb_d[j * QH:(j + 1) * QH].rearrange("(jj p) -> p jj", p=P))

            # ---- s_k = rsqrt(nk2) with one Newton step (column layout) ----
            sk_y = pp.tile([P, NKT], f32, name="sk_y")
            nc.scalar.sqrt(sk_y[:], nk2c[:])
            sk_r0 = pp.tile([P, NKT], f32, name="sk_r0")
            nc.vector.reciprocal(sk_r0[:], sk_y[:])
            sk_t1 = pp.tile([P, NKT], f32, name="sk_t1")
            nc.vector.tensor_mul(sk_t1[:], sk_r0[:], sk_r0[:])
            sk_t2 = pp.tile([P, NKT], f32, name="sk_t2")
            nc.vector.tensor_mul(sk_t2[:], sk_t1[:], nk2c[:])
            sk_t3 = pp.tile([P, NKT], f32, name="sk_t3")
            nc.vector.tensor_scalar(
                sk_t3[:], sk_t2[:], -0.5, 1.5, AluOpType.mult, AluOpType.add)
            sk_c = pp.tile([P, NKT], f32, name="sk_c")
            nc.vector.tensor_mul(sk_c[:], sk_t3[:], sk_r0[:])

            # mask bias columns: (m - 1) * 1e9  -> 0 (keep) / -1e9 (masked)
            biasc = pp.tile([P, NKT], f32, name="biasc")
            nc.vector.tensor_scalar(
                biasc[:], maskc[:], 1.0, 1e9, AluOpType.subtract, AluOpType.mult)

            # ---- main matmul: out_T[k, q] ----
            for kt in range(NKT):
                pos = []
                for h in range(NQH):
                    po = pso.tile([P, QH], f32, name="po", tag="po")
                    pos.append(po)
                for d in range(ND):
                    for h in range(NQH):
                        nc.tensor.matmul(
                            pos[h][:],
                            kt_ch[d][:, kt * P:(kt + 1) * P],
                            qts_ch[d][:, h * QH:(h + 1) * QH],
                            start=(d == 0), stop=(d == ND - 1))
                for h in range(NQH):
                    ot = outp.tile([P, QH], f32, name="ot", tag="ot")
                    nc.scalar.activation(
                        ot[:], pos[h][:], AF.Identity,
                        bias=biasc[:, kt:kt + 1], scale=sk_c[:, kt:kt + 1])
                    nc.sync.dma_start(
                        out_d[kt * P:(kt + 1) * P, h * QH:(h + 1) * QH], ot[:])

    nc.compile()
    return nc


def _get_nc():
    key = (SQ, SK, D)
    if key not in _NC_CACHE:
        _NC_CACHE[key] = build_nc()
    return _NC_CACHE[key]


def kernel(query, key, mask):
    from concourse import bass_utils

    query = np.asarray(query, dtype=np.float32)
    key = np.asarray(key, dtype=np.float32)
    mask_np = np.asarray(mask)

    nc = _get_nc()

    in_maps = []
    for c in range(N_CORES):
        b, h = c // 2, c % 2
        q = query[b, h * SQ:(h + 1) * SQ, 0, :]          # [SQ, D]
        k = key[b, 0, :, :]                              # [SK, D]
        in_maps.append({
            "qt": np.ascontiguousarray(q.T).astype(ml_dtypes.bfloat16),
            "kt": np.ascontiguousarray(k.T).astype(ml_dtypes.bfloat16),
            "maskf": mask_np[b].astype(np.float32),
        })

    trace = bool(int(os.environ.get("KERNEL_TRACE", "0")))
    res = bass_utils.run_bass_kernel_spmd(
        nc, in_maps, core_ids=list(range(N_CORES)), trace=trace)
    kernel.last_results = res

    out = np.empty((B, SQ_FULL, SK), np.float32)
    for c in range(N_CORES):
        b, h = c // 2, c % 2
        out[b, h * SQ:(h + 1) * SQ, :] = res.results[c]["out"].T
    return out


# revision 8
# speedup vs baseline: 1.0144x; 1.0144x over previous
"""Masked cosine-similarity attention scores on 8 trn2 NeuronCores.

Problem (per full inputs):
    query [B=4, Sq=2048, 1, D=1024] f32
    key   [B=4, 1, Sk=2048, D=1024] f32
    mask  [B=4, Sk=2048] int32 (0/1)
    out[b,q,k] = (q.k)/(max(|q|,eps)*max(|k|,eps)),  -1e9 where mask[b,k]==0

Sharding: 8 cores = (batch b, q-half h); each core computes the TRANSPOSED
output tile out_T [Sk=2048, Sq_loc=1024] for its (b, h).

Per-core device algorithm (all math on device):
  - Q^T, K^T arrive in [D, rows] bf16 layout (host does layout/dtype prep only).
  - row norms^2 via squares + ones-matmul partition reduction on the PE.
  - 1/norm via ACT Sqrt + DVE reciprocal + one Newton step (rel err ~1e-5).
  - q-scales broadcast to all partitions via ones outer-product matmul, then
    folded into Q^T on the DVE.
  - main matmul: out_T[k,q] accumulated over 8 d-chunks in PSUM (bf16 PE).
  - k-scale and mask bias (-1e9) fused into the PSUM->SBUF eviction as
    ACT activation(Identity, scale=s_k[P,1], bias=bias[P,1]); masked entries
    come out as exactly -1e9 in fp32.
"""

import os
import sys

import numpy as np

for _p in ("/opt/trn_rl_repo", "/opt/pypackages"):
    if _p not in sys.path and os.path.isdir(_p):
        sys.path.append(_p)

import ml_dtypes  # noqa: E402

_NC_CACHE = {}

# Full-problem constants (hardcoded per harness contract)
B, SQ_FULL, SK, D = 4, 2048, 2048, 1024
N_CORES = 8
SQ = SQ_FULL * B // N_CORES  # 1024 local q rows per core
P = 128


def build_nc(SQ=SQ, SK=SK, D=D, QH=512):
    """Build the single-core Bass program (SPMD: same program, per-core data)."""
    import concourse.mybir as mybir
    from concourse import bacc
    from concourse.alu_op_type import AluOpType
    from concourse.tile import TileContext

    f32 = mybir.dt.float32
    bf16 = mybir.dt.bfloat16
    AF = mybir.ActivationFunctionType

    ND = D // P       # d-chunks
    NKT = SK // P     # k-tiles (output partition tiles)
    NQH = SQ // QH    # q column chunks
    NKC = SK // QH    # k column chunks for norm reduce

    nc = bacc.Bacc("TRN2", target_bir_lowering=False, debug=False)
    qt_d = nc.declare_dram_parameter("qt", [D, SQ], bf16, isOutput=False)
    kt_d = nc.declare_dram_parameter("kt", [D, SK], bf16, isOutput=False)
    mk_d = nc.declare_dram_parameter("maskf", [SK], f32, isOutput=False)
    out_d = nc.declare_dram_parameter("out", [SK, SQ], f32, isOutput=True)
    nkb_d = nc.dram_tensor("nk_bounce", [SK], f32)

    with TileContext(nc) as tc:
        with (
            tc.tile_pool(name="pp", bufs=1) as pp,
            tc.tile_pool(name="rows", bufs=3) as rows,
            tc.tile_pool(name="outp", bufs=4) as outp,
            tc.tile_pool(name="psA", bufs=2, space="PSUM") as psA,
            tc.tile_pool(name="psK", bufs=1, space="PSUM") as psK,
            tc.tile_pool(name="pso", bufs=5, space="PSUM") as pso,
        ):
            # ---- constants ----
            ones_bf = pp.tile([P, 1], bf16, name="ones_bf")
            nc.vector.memset(ones_bf[:], 1.0)
            ones_f1 = pp.tile([1, P], f32, name="ones_f1")
            nc.vector.memset(ones_f1[:], 1.0)

            # ---- input DMAs (interleaved so both streams arrive early) ----
            qt_ch, kt_ch = [], []
            for d in range(ND):
                tq = pp.tile([P, SQ], bf16, name=f"qtc{d}", tag=f"qtc{d}")
                nc.sync.dma_start(tq[:], qt_d[d * P:(d + 1) * P, :])
                qt_ch.append(tq)
                tk = pp.tile([P, SK], bf16, name=f"ktc{d}", tag=f"ktc{d}")
                nc.sync.dma_start(tk[:], kt_d[d * P:(d + 1) * P, :])
                kt_ch.append(tk)
            maskc = pp.tile([P, NKT], f32, name="maskc")
            nc.sync.dma_start(maskc[:], mk_d.rearrange("(j p) -> p j", p=P))

            # ---- q norms: squares (DVE) + ones-matmul partition reduce (PE) ----
            qsq_ch = []
            for d in range(ND):
                t = pp.tile([P, SQ], bf16, name=f"qsq{d}", tag=f"qsq{d}")
                nc.vector.tensor_mul(t[:], qt_ch[d][:], qt_ch[d][:])
                qsq_ch.append(t)
            nq2r = rows.tile([1, SQ], f32, name="nq2r", tag="qrowA")
            for j in range(NQH):
                npq = psA.tile([1, QH], f32, name="npq", tag="psA")
                for d in range(ND):
                    nc.tensor.matmul(
                        npq[:], ones_bf[:], qsq_ch[d][:, j * QH:(j + 1) * QH],
                        start=(d == 0), stop=(d == ND - 1))
                nc.vector.tensor_copy(nq2r[0:1, j * QH:(j + 1) * QH], npq[:])

            # ---- s_q = rsqrt(nq2) with one Newton step (on partition-0 rows) ----
            sq_y = rows.tile([1, SQ], f32, name="sq_y", tag="qrowB")
            nc.scalar.sqrt(sq_y[:], nq2r[:])
            sq_r0 = rows.tile([1, SQ], f32, name="sq_r0", tag="qrowC")
            nc.vector.reciprocal(sq_r0[:], sq_y[:])
            sq_t1 = rows.tile([1, SQ], f32, name="sq_t1", tag="qrowB")
            nc.vector.tensor_mul(sq_t1[:], sq_r0[:], sq_r0[:])
            sq_t2 = rows.tile([1, SQ], f32, name="sq_t2", tag="qrowA")
            nc.vector.tensor_mul(sq_t2[:], sq_t1[:], nq2r[:])
            sq_t3 = rows.tile([1, SQ], f32, name="sq_t3", tag="qrowB")
            nc.vector.tensor_scalar(
                sq_t3[:], sq_t2[:], -0.5, 1.5, AluOpType.mult, AluOpType.add)
            sq_row = rows.tile([1, SQ], f32, name="sq_row", tag="qrowA")
            nc.vector.tensor_mul(sq_row[:], sq_t3[:], sq_r0[:])

            # ---- broadcast s_q across partitions (kept f32, applied at
            # eviction so the main matmul never waits on the q-norm chain) ----
            sq_bc = pp.tile([P, SQ], f32, name="sq_bc")
            for j in range(NQH):
                pb = psA.tile([P, QH], f32, name="pb", tag="psA")
                nc.tensor.matmul(
                    pb[:], ones_f1[:], sq_row[0:1, j * QH:(j + 1) * QH],
                    start=True, stop=True)
                nc.vector.tensor_copy(sq_bc[:, j * QH:(j + 1) * QH], pb[:])

            # ---- k norms: squares (DVE) + ones-matmul reduce (PE) ----
            ksq_ch = []
            for d in range(ND):
                t = pp.tile([P, SK], bf16, name=f"ksq{d}", tag=f"ksq{d}")
                nc.vector.tensor_mul(t[:], kt_ch[d][:], kt_ch[d][:])
                ksq_ch.append(t)
            # Row norms^2 per 512-chunk -> DRAM bounce -> column layout.
            # Each DMA is kept to a single producer (HW limit on sync waits).
            nk2c = pp.tile([P, NKT], f32, name="nk2c")
            JPC = QH // P  # columns of nk2c covered per 512-chunk
            for j in range(NKC):
                npk = psK.tile([1, QH], f32, name="npk", tag="psK")
                for d in range(ND):
                    nc.tensor.matmul(
                        npk[:], ones_bf[:], ksq_ch[d][:, j * QH:(j + 1) * QH],
                        start=(d == 0), stop=(d == ND - 1))
                nkr = rows.tile([1, QH], f32, name="nkr", tag=f"krow{j}")
                nc.vector.tensor_copy(nkr[:], npk[:])
                nc.sync.dma_start(nkb_d[j * QH:(j + 1) * QH], nkr[0:1, :])
                nc.sync.dma_start(
                    nk2c[:, j * JPC:(j + 1) * JPC],
                    nkb_d[j * QH:(j + 1) * QH].rearrange("(jj p) -> p jj", p=P))

            # ---- s_k = rsqrt(nk2) with one Newton step (column layout) ----
            sk_y = pp.tile([P, NKT], f32, name="sk_y")
            nc.scalar.sqrt(sk_y[:], nk2c[:])
            sk_r0 = pp.tile([P, NKT], f32, name="sk_r0")
            nc.vector.reciprocal(sk_r0[:], sk_y[:])
            sk_t1 = pp.tile([P, NKT], f32, name="sk_t1")
            nc.vector.tensor_mul(sk_t1[:], sk_r0[:], sk_r0[:])
            sk_t2 = pp.tile([P, NKT], f32, name="sk_t2")
            nc.vector.tensor_mul(sk_t2[:], sk_t1[:], nk2c[:])
            sk_t3 = pp.tile([P, NKT], f32, name="sk_t3")
            nc.vector.tensor_scalar(
                sk_t3[:], sk_t2[:], -0.5, 1.5, AluOpType.mult, AluOpType.add)
            sk_c = pp.tile([P, NKT], f32, name="sk_c")
            nc.vector.tensor_mul(sk_c[:], sk_t3[:], sk_r0[:])

            # mask bias columns: (m - 1) * 1e9  -> 0 (keep) / -1e9 (masked)
            biasc = pp.tile([P, NKT], f32, name="biasc")
            nc.vector.tensor_scalar(
                biasc[:], maskc[:], 1.0, 1e9, AluOpType.subtract, AluOpType.mult)

            # ---- main matmul: out_T[k, q] over raw bf16 Q^T/K^T ----
            for kt in range(NKT):
                pos = []
                for h in range(NQH):
                    po = pso.tile([P, QH], f32, name="po", tag="po")
                    pos.append(po)
                for d in range(ND):
                    for h in range(NQH):
                        nc.tensor.matmul(
                            pos[h][:],
                            kt_ch[d][:, kt * P:(kt + 1) * P],
                            qt_ch[d][:, h * QH:(h + 1) * QH],
                            start=(d == 0), stop=(d == ND - 1))
                for h in range(NQH):
                    # (psum * s_k) * s_q_bcast on DVE, then + mask bias on ACT
                    ev = outp.tile([P, QH], f32, name="ev", tag="ev")
                    nc.vector.scalar_tensor_tensor(
                        ev[:], pos[h][:], sk_c[:, kt:kt + 1],
                        sq_bc[:, h * QH:(h + 1) * QH],
                        AluOpType.mult, AluOpType.mult)
                    ot = outp.tile([P, QH], f32, name="ot", tag="ot")
                    nc.scalar.activation(
                        ot[:], ev[:], AF.Identity,
                        bias=biasc[:, kt:kt + 1], scale=1.0)
                    nc.sync.dma_start(
                        out_d[kt * P:(kt + 1) * P, h * QH:(h + 1) * QH], ot[:])

    nc.compile()
    return nc


def _get_nc():
    key = (SQ, SK, D)
    if key not in _NC_CACHE:
        _NC_CACHE[key] = build_nc()
    return _NC_CACHE[key]


def kernel(query, key, mask):
    from concourse import bass_utils

    query = np.asarray(query, dtype=np.float32)
    key = np.asarray(key, dtype=np.float32)
    mask_np = np.asarray(mask)

    nc = _get_nc()

    in_maps = []
    for c in range(N_CORES):
        b, h = c // 2, c % 2
        q = query[b, h * SQ:(h + 1) * SQ, 0, :]          # [SQ, D]
        k = key[b, 0, :, :]                              # [SK, D]
        in_maps.append({
            "qt": np.ascontiguousarray(q.T).astype(ml_dtypes.bfloat16),
            "kt": np.ascontiguousarray(k.T).astype(ml_dtypes.bfloat16),
            "maskf": mask_np[b].astype(np.float32),
        })

    trace = bool(int(os.environ.get("KERNEL_TRACE", "0")))
    res = bass_utils.run_bass_kernel_spmd(
        nc, in_maps, core_ids=list(range(N_CORES)), trace=trace)
    kernel.last_results = res

    out = np.empty((B, SQ_FULL, SK), np.float32)
    for c in range(N_CORES):
        b, h = c // 2, c % 2
        out[b, h * SQ:(h + 1) * SQ, :] = res.results[c]["out"].T
    return out
